# revision 6
# baseline (speedup 1.0000x reference)
"""MoE expert-fusion kernel for Trainium2 (8 NeuronCores).

Math: reference computes expert_out[e] = x @ W_e + b_e for ALL E=8 experts,
then contracts with top-k-masked scores. Only k=2 experts have nonzero
weight per batch row, and the contraction is linear in W, so:

    out[b] = x[b] @ (s0*W_{e0} + s1*W_{e1}) + (s0*b_{e0} + s1*b_{e1})

Sharding: one batch row per core (B=8 = n_cores). Host does the top-k
routing (data-dependent *sharding* decision: which expert weights get
shipped to which core) and ships x[b].T plus the two selected expert
weights. The device combines the weights (VectorE) and runs the single
2048x1024x1024 matmul (TensorE, fp32r) with fused bias epilogue.
"""

import numpy as np

import concourse.bass as bass
import concourse.mybir as mybir
from concourse.tile import TileContext

# ---------------------------------------------------------------------------
# Problem constants (hardcoded per harness contract)
B, S, D, E = 8, 2048, 1024, 8
N_CORES = 8
P = 128          # SBUF partitions
KC = D // P      # contraction chunks (8)
FT = D // P      # output-feature tiles of 128 (8)
QW = 512         # moving free-dim per matmul (PSUM bank limit for fp32)
Q = S // QW      # token quarters (4)
F32 = mybir.dt.float32
F32R = mybir.dt.float32r


_MAX_WAITS = 1  # this walrus build rejects instructions with >1 sync wait


def _split_sync_waits(nc: bass.Bass) -> None:
    """Walrus in this toolchain caps sync waits at 2 per instruction, but
    Tile emits up to ~3 (and piles the whole global clock on the tail
    drain). Move excess waits onto InstNoOp instructions inserted just
    before the offending instruction on the same engine — engine streams
    execute in order, so semantics are identical."""
    serial = 0
    for fn in nc.m.functions:
        for blk in fn.blocks:
            il = blk.instructions
            i = 0
            while i < len(il):
                inst = il[i]
                si = inst.sync_info
                if si is not None and si.on_wait and len(si.on_wait) > _MAX_WAITS:
                    waits = list(si.on_wait)
                    si.on_wait = waits[-_MAX_WAITS:]
                    extra = waits[: -_MAX_WAITS]
                    pos = i
                    for j in range(0, len(extra), _MAX_WAITS):
                        nop = mybir.InstNoOp(name=f"W-split-{serial}")
                        serial += 1
                        nop.engine = inst.engine
                        nop.sync_info = mybir.SyncInfo(
                            on_wait=extra[j : j + _MAX_WAITS], on_update=[]
                        )
                        il.insert(pos, nop)
                        pos += 1
                        i += 1
                i += 1


def build_graph(k_active: int = 2) -> bass.Bass:
    """Per-core graph. Inputs (per core / shard):
      xt [D, S]            x[b].T (contraction dim leading -> natural lhs layout)
      ws [k, D, D]         selected expert weight matrices
      sc [P, k]            active masked scores, broadcast over partitions
      bt [P, k, FT]        selected biases, bt[p, j, f] = b_j[f*128 + p]
    Output:
      out [D, S]           out[b].T (host transposes back)
    """
    nc = bass.Bass()
    xt = nc.declare_dram_parameter("xt", [D, S], F32R, isOutput=False)
    ws = nc.declare_dram_parameter("ws", [k_active, D, D], F32, isOutput=False)
    sc = nc.declare_dram_parameter("sc", [P, k_active], F32, isOutput=False)
    bt = nc.declare_dram_parameter("bt", [P, k_active, FT], F32, isOutput=False)
    out = nc.declare_dram_parameter("out", [D, S], F32, isOutput=True)

    xt_r = xt.rearrange("(c p) t -> c p t", p=P)          # [KC, P, S]
    MULT = mybir.AluOpType.mult
    ADD = mybir.AluOpType.add

    with TileContext(nc) as tc:
        with (
            tc.tile_pool(name="const", bufs=1) as cpool,
            tc.tile_pool(name="xs", bufs=1) as xpool,
            tc.tile_pool(name="wcs", bufs=1) as wcpool,
            tc.tile_pool(name="wstream", bufs=3) as wpool,
            tc.tile_pool(name="outs", bufs=6) as opool,
            tc.tile_pool(name="ps", bufs=8, space="PSUM") as pspool,
        ):
            sc_s = cpool.tile([P, k_active], F32, tag="sc")
            nc.sync.dma_start(out=sc_s[:], in_=sc[:])
            bt_s = cpool.tile([P, k_active, FT], F32, tag="bt")
            nc.sync.dma_start(out=bt_s[:], in_=bt[:])

            # bc[:, f] = sum_j s_j * b_j[f*128 + p]
            bc_s = cpool.tile([P, FT], F32, tag="bc")
            nc.vector.tensor_scalar_mul(bc_s[:], bt_s[:, 0, :], sc_s[:, 0:1])
            for j in range(1, k_active):
                nc.vector.scalar_tensor_tensor(
                    bc_s[:], bt_s[:, j, :], sc_s[:, j : j + 1], bc_s[:], MULT, ADD
                )

            # x^T chunks: x_s[p, c, t] = x[b].T[c*128+p, t]
            x_s = xpool.tile([P, KC, S], F32R, tag="x")
            for c in range(KC):
                nc.sync.dma_start(out=x_s[:, c, :], in_=xt_r[c])

            # combined weights: wc[p, c, f] = sum_j s_j * W_j[c*128+p, f]
            wc_s = wcpool.tile([P, KC, D], F32R, tag="wc")
            for c in range(KC):
                rows = slice(c * P, (c + 1) * P)
                w0c = wpool.tile([P, D], F32, tag="w0c")
                nc.sync.dma_start(out=w0c[:], in_=ws[0, rows, :])
                if k_active == 1:
                    nc.vector.tensor_scalar_mul(wc_s[:, c, :], w0c[:], sc_s[:, 0:1])
                    continue
                acc = wpool.tile([P, D], F32, tag="wacc")
                nc.vector.tensor_scalar_mul(acc[:], w0c[:], sc_s[:, 0:1])
                for j in range(1, k_active):
                    wjc = wpool.tile([P, D], F32, tag="w1c")
                    nc.sync.dma_start(out=wjc[:], in_=ws[j, rows, :])
                    dst = wc_s[:, c, :] if j == k_active - 1 else acc[:]
                    nc.vector.scalar_tensor_tensor(
                        dst, wjc[:], sc_s[:, j : j + 1], acc[:], MULT, ADD
                    )

            # out^T[f*128+p, q*512+t] = sum_c wc[:, c, f-slice].T @ x[:, c, q-slice]
            for q in range(Q):
                cols = slice(q * QW, (q + 1) * QW)
                ps = [
                    pspool.tile([P, QW], F32, name=f"ps_{q}_{f}", tag="ps")
                    for f in range(FT)
                ]
                for c in range(KC):
                    for f in range(FT):
                        nc.tensor.matmul(
                            ps[f][:],
                            wc_s[:, c, f * P : (f + 1) * P],
                            x_s[:, c, cols],
                            start=(c == 0),
                            stop=(c == KC - 1),
                        )
                for f in range(FT):
                    o = opool.tile([P, QW], F32, tag="o")
                    nc.vector.tensor_scalar_add(o[:], ps[f][:], bc_s[:, f : f + 1])
                    nc.sync.dma_start(out=out[f * P : (f + 1) * P, cols], in_=o[:])
    _split_sync_waits(nc)
    return nc


_GRAPH_CACHE: dict = {}


def _get_graph(k_active: int) -> bass.Bass:
    if k_active not in _GRAPH_CACHE:
        _GRAPH_CACHE[k_active] = build_graph(k_active)
    return _GRAPH_CACHE[k_active]


def _routing(routing_scores: np.ndarray, k: int):
    """Replicates jax.lax.top_k (descending, ties -> lower index) + the
    reference's renormalized gating, in float32."""
    scores = routing_scores.astype(np.float32)
    order = np.argsort(-scores, axis=1, kind="stable")
    idx = order[:, :k]                                       # (B, k)
    vals = np.take_along_axis(scores, idx, axis=1)           # (B, k)
    scale = np.float32(1.0) / (vals.sum(axis=1) + np.float32(1e-8))
    s = (scale[:, None] * vals).astype(np.float32)           # (B, k)
    counts = np.zeros(scores.shape[1], np.int32)
    for b in range(idx.shape[0]):
        counts[idx[b]] += 1
    return idx, s, counts


def _make_in_maps(x, routing_scores, expert_w, expert_b, k):
    idx, s, counts = _routing(routing_scores, k)
    in_maps = []
    for b in range(B):
        sel = idx[b]
        ws = np.ascontiguousarray(expert_w[sel])                    # [k, D, D]
        xt = np.ascontiguousarray(x[b].T)                           # [D, S]
        sc = np.broadcast_to(s[b], (P, k)).copy()                   # [P, k]
        bsel = expert_b[sel]                                        # [k, D]
        bt = np.ascontiguousarray(
            bsel.reshape(k, FT, P).transpose(2, 0, 1)               # [P, k, FT]
        )
        in_maps.append({"xt": xt, "ws": ws, "sc": sc, "bt": bt})
    return in_maps, counts


def run(x, routing_scores, expert_w, expert_b, k, trace=False, **spmd_kwargs):
    from concourse.bass_utils import run_bass_kernel_spmd

    k = int(k)
    nc = _get_graph(k)
    in_maps, counts = _make_in_maps(x, routing_scores, expert_w, expert_b, k)
    res = run_bass_kernel_spmd(
        nc, in_maps, core_ids=list(range(N_CORES)), trace=trace, **spmd_kwargs
    )
    final = np.empty((B, S, D), np.float32)
    for b in range(B):
        final[b] = res.results[b]["out"].T
    return (final, counts), res


def kernel(x, routing_scores, expert_w, expert_b, k):
    (final, counts), _ = run(
        np.asarray(x), np.asarray(routing_scores),
        np.asarray(expert_w), np.asarray(expert_b), k,
    )
    return final, counts
